# revision 1
# baseline (speedup 1.0000x reference)
"""Trainium2 Bass kernel for nn_CCM: per-pixel complex 3x3 conv mask.

Math (per batch element b, sharded 1 batch element per NeuronCore):
  y[t,f] = sum_{c=0..26} m[c,t,f] * (w_{k(c)} * X)[t+i(c)-2, f+j(c)-1]
where c = 9*k + 3*i + j, w_k = v[0,k] + 1j*v[1,k] (cube roots of unity),
X = xr + 1j*xi, zero padded (causal in t: 2 top; symmetric in f: 1,1).

Layout: t = 8*p + tau, partitions p in [0,125), (tau, f) in the free dim,
so every tap shift is a free-dim offset read of padded "U" planes
U_k = w_k * X stored as [125, 10 tau-slots, 259 f-cols] (slots tau=-2..7).
"""

import sys
import numpy as np

sys.path.insert(0, "/opt/trn_rl_repo")

B = 8
C = 27
T = 1000
F = 257
TP = 125          # partitions
TAU = 8           # t = 8*p + tau
NS = 10           # tau slots in U planes: tau in [-2, 8)
FP = 259          # padded f width: f in [-1, 258)
SQ3H = float(np.sqrt(3.0) / 2.0)

_CACHE = {}


def _emit(ctx, tc, m_ap, x_ap, id_ap, y_ap):
    import concourse.mybir as mybir

    nc = tc.nc
    f32 = mybir.dt.float32
    FCS = [(0, 128), (128, 128), (256, 1)]  # f chunks for transposes

    const = ctx.enter_context(tc.tile_pool(name="const", bufs=1))
    planes = ctx.enter_context(tc.tile_pool(name="planes", bufs=1))
    mpool = ctx.enter_context(tc.tile_pool(name="mtiles", bufs=3))
    work = ctx.enter_context(tc.tile_pool(name="work", bufs=3))
    psum = ctx.enter_context(tc.tile_pool(name="psum", bufs=3, space="PSUM"))

    ident = const.tile([128, 128], f32, tag="ident")
    nc.sync.dma_start(ident[:], id_ap)

    # ---- load x in natural layout: [f, (tt, comp)] with tt = t + 2 (2 zero rows)
    xns = []
    for (f0, fw) in FCS:
        xn = const.tile([fw, (T + 2) * 2], f32, tag=f"xn{f0}")
        nc.vector.memset(xn[:, 0:4], 0.0)
        nc.sync.dma_start(
            xn[:, 4:], x_ap[f0:f0 + fw].rearrange("f t c -> f (t c)")
        )
        xns.append(xn)

    # ---- transpose x into blocked padded planes xr, xi: [TP, NS, FP]
    xq = []
    for q in range(2):
        p = planes.tile([TP, NS, FP], f32, tag=f"xq{q}")
        nc.vector.memset(p[:], 0.0)
        xq.append(p)
    for q in range(2):
        for ts in range(NS):  # slot ts corresponds to tau = ts - 2; tt = 8p + ts
            for ci, (f0, fw) in enumerate(FCS):
                pt = psum.tile([TP, 128], f32, tag="tp")
                xn3 = xns[ci].rearrange("f (t c) -> f t c", c=2)
                nc.tensor.transpose(
                    pt[0:TP, 0:fw],
                    xn3[0:fw, ts:ts + TAU * (TP - 1) + 1:TAU, q],
                    ident[0:fw, 0:fw],
                )
                nc.scalar.copy(xq[q][:, ts, 1 + f0:1 + f0 + fw], pt[0:TP, 0:fw])

    # ---- U planes: U_k = w_k * (xr + i xi), w_k = exp(+-2pi i/3), w_0 = 1
    mult = mybir.AluOpType.mult
    add = mybir.AluOpType.add
    sub = mybir.AluOpType.subtract
    t1 = planes.tile([TP, NS, FP], f32, tag="t1")
    t2 = planes.tile([TP, NS, FP], f32, tag="t2")
    ur1 = planes.tile([TP, NS, FP], f32, tag="ur1")
    ui1 = planes.tile([TP, NS, FP], f32, tag="ui1")
    ur2 = planes.tile([TP, NS, FP], f32, tag="ur2")
    ui2 = planes.tile([TP, NS, FP], f32, tag="ui2")
    nc.vector.tensor_scalar_mul(t1[:], xq[1][:], SQ3H)  # xi * s
    nc.vector.tensor_scalar_mul(t2[:], xq[0][:], SQ3H)  # xr * s
    nc.vector.scalar_tensor_tensor(ur1[:], xq[0][:], -0.5, t1[:], op0=mult, op1=sub)
    nc.vector.scalar_tensor_tensor(ui1[:], xq[1][:], -0.5, t2[:], op0=mult, op1=add)
    nc.vector.scalar_tensor_tensor(ur2[:], xq[0][:], -0.5, t1[:], op0=mult, op1=add)
    nc.vector.scalar_tensor_tensor(ui2[:], xq[1][:], -0.5, t2[:], op0=mult, op1=sub)
    U = [(xq[0], xq[1]), (ur1, ui1), (ur2, ui2)]

    # ---- tap loop: acc += m_c * U_k[shifted]
    acc_r = planes.tile([TP, TAU, F], f32, tag="accr")
    acc_i = planes.tile([TP, TAU, F], f32, tag="acci")
    for c in range(C):
        kk, n = divmod(c, 9)
        i, j = divmod(n, 3)
        dt, df = i - 2, j - 1
        mt = mpool.tile([TP, TAU * F], f32, tag="mt")
        nc.sync.dma_start(mt[:], m_ap[c].rearrange("(p t) f -> p (t f)", p=TP))
        m3 = mt.rearrange("p (t f) -> p t f", f=F)
        ur, ui = U[kk]
        urs = ur[:, dt + 2:dt + 2 + TAU, df + 1:df + 1 + F]
        uis = ui[:, dt + 2:dt + 2 + TAU, df + 1:df + 1 + F]
        if c == 0:
            nc.vector.tensor_mul(acc_r[:], m3[:], urs)
            nc.vector.tensor_mul(acc_i[:], m3[:], uis)
        else:
            pr = work.tile([TP, TAU, F], f32, tag="prod")
            nc.vector.tensor_mul(pr[:], m3[:], urs)
            nc.vector.tensor_add(acc_r[:], acc_r[:], pr[:])
            pi = work.tile([TP, TAU, F], f32, tag="prod")
            nc.vector.tensor_mul(pi[:], m3[:], uis)
            nc.vector.tensor_add(acc_i[:], acc_i[:], pi[:])

    # ---- transpose back to [f, (t, comp)] and store
    for ci, (f0, fw) in enumerate(FCS):
        yo = const.tile([fw, T * 2], f32, tag=f"yo{f0}")
        yv = yo.rearrange("f (t c) -> f t c", c=2)
        for comp, acc in ((0, acc_r), (1, acc_i)):
            for ts in range(TAU):
                pt = psum.tile([128, TP], f32, tag="tp2")
                nc.tensor.transpose(
                    pt[0:fw, 0:TP], acc[:, ts, f0:f0 + fw], ident[0:TP, 0:TP]
                )
                nc.scalar.copy(
                    yv[0:fw, ts:ts + TAU * (TP - 1) + 1:TAU, comp], pt[0:fw, 0:TP]
                )
        nc.sync.dma_start(y_ap[f0:f0 + fw].rearrange("f t c -> f (t c)"), yo[:])


def _build():
    if "nc" in _CACHE:
        return _CACHE["nc"]
    from contextlib import ExitStack
    from concourse import bacc, mybir
    import concourse.tile as tile

    f32 = mybir.dt.float32
    nc = bacc.Bacc("TRN2", target_bir_lowering=False, debug=False, num_devices=B)
    m_d = nc.dram_tensor("m", (C, T, F), f32, kind="ExternalInput")
    x_d = nc.dram_tensor("x", (F, T, 2), f32, kind="ExternalInput")
    id_d = nc.dram_tensor("ident", (128, 128), f32, kind="ExternalInput")
    y_d = nc.dram_tensor("y", (F, T, 2), f32, kind="ExternalOutput")

    with tile.TileContext(nc) as tc:
        with ExitStack() as ctx:
            _emit(ctx, tc, m_d.ap(), x_d.ap(), id_d.ap(), y_d.ap())
    nc.compile()
    _CACHE["nc"] = nc
    return nc


def _in_maps(m, x):
    ident = np.eye(128, dtype=np.float32)
    return [
        {"m": np.ascontiguousarray(m[b]), "x": np.ascontiguousarray(x[b]),
         "ident": ident}
        for b in range(B)
    ]


def kernel(m, x, v, _trace=False):
    from concourse import bass_utils

    m = np.asarray(m, dtype=np.float32)
    x = np.asarray(x, dtype=np.float32)
    nc = _build()
    res = bass_utils.run_bass_kernel_spmd(
        nc, _in_maps(m, x), core_ids=list(range(B)), trace=_trace
    )
    kernel.last_results = res
    y = np.stack([res.results[b]["y"] for b in range(B)], axis=0)
    return y



# revision 2
# speedup vs baseline: 1.5062x; 1.5062x over previous
"""Trainium2 Bass kernel for nn_CCM: per-pixel complex 3x3 conv mask.

Math (per batch element b, 1 batch element per NeuronCore):
  y[t,f] = sum_{c=0..26} m[c,t,f] * U_{k(c)}[t+i(c)-2, f+j(c)-1]
where c = 9*k + 3*i + j, U_k = (v[0,k] + 1j*v[1,k]) * (xr + 1j*xi),
zero padded (causal in t: 2 top; symmetric in f: 1,1).

Device does ONLY the 27-tap MAC loop in fp16 (DVE 2x_1p mode, with a few
taps offloaded to GpSimd). All layout work happens on the host:
  - m pre-packed to [128, 27, 2056] fp16: partition p holds t rows
    8p..8p+7 (t = 8p + tau), flattened (tau, f).
  - U planes precomputed as [128, 6, 10, 259] fp16: plane 2k/2k+1 =
    real/imag of U_k; row slot ts covers t = 8p + ts - 2; col = f + 1.
  - Output acc planes [128, 2, 8, 257] fp16 unpacked/cast on host.
"""

import sys
import numpy as np

sys.path.insert(0, "/opt/trn_rl_repo")

B = 8
C = 27
T = 1000
F = 257
TP = 125          # real partitions (t = 8*p + tau)
NP = 128          # padded partition dim
TAU = 8
NS = 10           # tau slots in U planes: t offsets -2..7
FP = 259          # padded f width: f in [-1, 258)

# Taps executed on GpSimd (both real+imag mul+add); rest on DVE.
GP_TAPS = (0, 5, 11, 16, 22)
CHUNK = 3         # taps per m DMA chunk

_CACHE = {}


def _emit(ctx, tc, m_ap, u_ap, y_ap):
    import concourse.mybir as mybir

    nc = tc.nc
    f16 = mybir.dt.float16

    const = ctx.enter_context(tc.tile_pool(name="const", bufs=1))
    mpool = ctx.enter_context(tc.tile_pool(name="mtiles", bufs=3))
    work = ctx.enter_context(tc.tile_pool(name="work", bufs=4))

    ut = const.tile([NP, 6, NS, FP], f16, tag="u")
    nc.sync.dma_start(ut[:], u_ap)

    acc_r = const.tile([NP, TAU, F], f16, tag="accr")
    acc_i = const.tile([NP, TAU, F], f16, tag="acci")
    gacc_r = const.tile([NP, TAU, F], f16, tag="gaccr")
    gacc_i = const.tile([NP, TAU, F], f16, tag="gacci")

    first = {True: True, False: True}  # first tap per engine (gp, dve)
    for c0 in range(0, C, CHUNK):
        ntap = min(CHUNK, C - c0)
        mt = mpool.tile([NP, ntap, TAU, F], f16, tag="mt")
        nc.sync.dma_start(
            mt[:], m_ap[:, c0:c0 + ntap].rearrange("p c (t f) -> p c t f", f=F)
        )
        for ci in range(ntap):
            c = c0 + ci
            kk, n = divmod(c, 9)
            i, j = divmod(n, 3)
            is_gp = c in GP_TAPS
            eng = nc.gpsimd if is_gp else nc.vector
            ar = gacc_r if is_gp else acc_r
            ai = gacc_i if is_gp else acc_i
            m3 = mt[:, ci]
            urs = ut[:, 2 * kk, i:i + TAU, j:j + F]
            uis = ut[:, 2 * kk + 1, i:i + TAU, j:j + F]
            if first[is_gp]:
                eng.tensor_mul(ar[:], m3, urs)
                eng.tensor_mul(ai[:], m3, uis)
                first[is_gp] = False
            else:
                pr = work.tile([NP, TAU, F], f16, tag="pr")
                eng.tensor_mul(pr[:], m3, urs)
                eng.tensor_add(ar[:], ar[:], pr[:])
                pi = work.tile([NP, TAU, F], f16, tag="pi")
                eng.tensor_mul(pi[:], m3, uis)
                eng.tensor_add(ai[:], ai[:], pi[:])

    if GP_TAPS:
        nc.vector.tensor_add(acc_r[:], acc_r[:], gacc_r[:])
        nc.vector.tensor_add(acc_i[:], acc_i[:], gacc_i[:])
    nc.sync.dma_start(y_ap[:, 0], acc_r[:])
    nc.sync.dma_start(y_ap[:, 1], acc_i[:])


def _build():
    if "nc" in _CACHE:
        return _CACHE["nc"]
    from contextlib import ExitStack
    from concourse import bacc, mybir
    import concourse.tile as tile

    f16 = mybir.dt.float16
    nc = bacc.Bacc("TRN2", target_bir_lowering=False, debug=False, num_devices=B)
    m_d = nc.dram_tensor("m", (NP, C, TAU * F), f16, kind="ExternalInput")
    u_d = nc.dram_tensor("u", (NP, 6, NS, FP), f16, kind="ExternalInput")
    y_d = nc.dram_tensor("y", (NP, 2, TAU, F), f16, kind="ExternalOutput")

    with tile.TileContext(nc) as tc:
        with ExitStack() as ctx:
            _emit(ctx, tc, m_d.ap(), u_d.ap(), y_d.ap())
    nc.compile()
    _CACHE["nc"] = nc
    return nc


def _prep_inputs(m, x, v):
    """Host-side packing: returns per-core input maps."""
    # m: (B, 27, 1000, 257) -> (B, 128, 27, 8*257) fp16, partition-blocked
    mT = np.zeros((B, NP, C, TAU * F), dtype=np.float16)
    mT[:, :TP] = (
        m.reshape(B, C, TP, TAU * F).transpose(0, 2, 1, 3).astype(np.float16)
    )

    # padded planes xr, xi: (B, 125, 10, 259) f32
    Xr = np.ascontiguousarray(x[..., 0].transpose(0, 2, 1))  # (B, T, F)
    Xi = np.ascontiguousarray(x[..., 1].transpose(0, 2, 1))
    xr = np.zeros((B, TP, NS, FP), dtype=np.float32)
    xi = np.zeros((B, TP, NS, FP), dtype=np.float32)
    for ts in range(NS):
        off = ts - 2
        p0 = 1 if off < 0 else 0
        # t = 8*p + off for p in [p0, 125); all <= 999 here
        xr[:, p0:, ts, 1:1 + F] = Xr[:, 8 * p0 + off::TAU, :][:, :TP - p0]
        xi[:, p0:, ts, 1:1 + F] = Xi[:, 8 * p0 + off::TAU, :][:, :TP - p0]

    u6 = np.zeros((B, NP, 6, NS, FP), dtype=np.float16)
    for k in range(3):
        u6[:, :TP, 2 * k] = (v[0, k] * xr - v[1, k] * xi).astype(np.float16)
        u6[:, :TP, 2 * k + 1] = (v[0, k] * xi + v[1, k] * xr).astype(np.float16)

    return [{"m": mT[b], "u": u6[b]} for b in range(B)]


def kernel(m, x, v, _trace=False):
    from concourse import bass_utils

    m = np.asarray(m, dtype=np.float32)
    x = np.asarray(x, dtype=np.float32)
    v = np.asarray(v, dtype=np.float32)
    nc = _build()
    res = bass_utils.run_bass_kernel_spmd(
        nc, _prep_inputs(m, x, v), core_ids=list(range(B)), trace=_trace
    )
    kernel.last_results = res
    # y device layout: (128, 2, 8, 257) fp16 -> (B, F, T, 2) f32
    out = np.empty((B, F, T, 2), dtype=np.float32)
    for b in range(B):
        acc = res.results[b]["y"][:TP].astype(np.float32)  # (125, 2, 8, 257)
        yr = acc[:, 0].reshape(T, F)
        yi = acc[:, 1].reshape(T, F)
        out[b] = np.stack([yr, yi], axis=2).transpose(1, 0, 2)
    return out
